# revision 5
# baseline (speedup 1.0000x reference)
"""MultiHeadAttention with RoPE on 8 Trainium2 NeuronCores.

Sharding: batch (2) x head-group (4 heads each) -> 8 cores. Each core
computes q/k/v projections for its 4 heads of one batch element, causal
attention, and a partial output projection (row-shard of Wo). The host
sums the 4 partial outputs per batch element (the "all-reduce").

v3: single fused pipeline to keep the PE dense (HAM warm) end to end.
  - Input DMA split per 512-token chunk (tt-major) so projections start
    ~2us in instead of after the full 7MiB load.
  - Per-tt stage: Q/K proj + rope for chunk tt, then attention stage
    qt=tt (scores/exp/AV for all kb), then out-projection for its 4
    token blocks.  V projection (token-major, xt stationary) runs first,
    double-buffered in its own scoped PSUM pool.
  - softmax denominators: reciprocal_approx_fast on the ones-row (was
    exact DVE reciprocal on a 1-partition AP = 3.4us each), then gpsimd
    partition_broadcast + DVE multiply.
  - rope done in bf16 off a psum->sbuf cast (2x DVE rate, early psum
    release); sin-mul on gpsimd.
  - causal diagonal mask via DVE multiply with a precomputed triangle
    tile (affine_select on gpsimd was sem-overhead heavy).
  - out-projection split per 512-col half (1-bank psum tiles), psum->
    sbuf eviction on ACT, store fp32.

PSUM budget: V(2, scoped) -> QK(2) + SC(2) + AV(2) + OP(2) = 8 banks.
"""

import numpy as np
import ml_dtypes

import concourse.bacc as bacc
import concourse.mybir as mybir
import concourse.tile as tile
from concourse.bass_utils import run_bass_kernel_spmd

F32 = mybir.dt.float32
BF16 = mybir.dt.bfloat16
EXP = mybir.ActivationFunctionType.Exp
LN = mybir.ActivationFunctionType.Ln

B, S, D = 2, 2048, 1024
H, HD = 16, 64
THETA = 10000.0
NCORES = 8
NH = 4          # heads per core
C = NH * HD     # 256 channels per core
P = 128
DC = D // P     # 8 contraction chunks
NQT = S // 512  # 4 q-tiles
NTB = S // P    # 16 token blocks

_NC_CACHE = None
LAST_RESULTS = None


def _build():
    nc = bacc.Bacc(None)

    xT = nc.dram_tensor("xT", [D, S], BF16, kind="ExternalInput")
    wqT = nc.dram_tensor("wqT", [D, C], BF16, kind="ExternalInput")
    wkT = nc.dram_tensor("wkT", [D, C], BF16, kind="ExternalInput")
    wvT = nc.dram_tensor("wvT", [D, C], BF16, kind="ExternalInput")
    woT = nc.dram_tensor("woT", [C, D], BF16, kind="ExternalInput")
    cosP = nc.dram_tensor("cosP", [P, S], BF16, kind="ExternalInput")
    sinP = nc.dram_tensor("sinP", [P, S], BF16, kind="ExternalInput")
    out = nc.dram_tensor("out", [S, D], F32, kind="ExternalOutput")

    xT3 = xT.rearrange("(dc di) t -> di dc t", di=P)
    woT3 = woT.rearrange("(cp ci) o -> ci cp o", ci=P)

    XOR1 = [i ^ 1 for i in range(32)]

    with tile.TileContext(nc) as tc:
        with (
            tc.tile_pool(name="cn", bufs=1) as cn,
            tc.tile_pool(name="big", bufs=1) as big,
            tc.tile_pool(name="psQK", bufs=2, space="PSUM") as psQK,
            tc.tile_pool(name="shp", bufs=3) as shp,
            tc.tile_pool(name="ex", bufs=6) as ex,
            tc.tile_pool(name="nrm", bufs=4) as nrm,
            tc.tile_pool(name="ob", bufs=3) as ob,
        ):
            # ---- long-lived tiles ----
            cos_sb = cn.tile([P, S], BF16, tag="cos")
            sin_sb = cn.tile([P, S], BF16, tag="sin")
            w_sb = {}
            for proj in ("v", "q", "k"):
                w_sb[proj] = cn.tile([P, DC, C], BF16, tag=f"w{proj}",
                                     name=f"w{proj}")
            xt_sb = cn.tile([P, DC, S], BF16, tag="xt")
            wo_sb = cn.tile([P, 2, D], BF16, tag="wo")
            # V in token-major blocks: [tok, tb, head, hd | ones]
            vp = cn.tile([P, NTB, NH, HD + 1], BF16, tag="vp")
            tri = cn.tile([P, P], BF16, tag="tri")   # keep q>=k mask

            qk = {}
            for proj in ("q", "k"):
                for pair in range(2):
                    qk[(proj, pair)] = big.tile(
                        [P, S], BF16, tag=f"{proj}{pair}", name=f"{proj}{pair}")
            yt = {pair: big.tile([P, S], BF16, tag=f"y{pair}", name=f"y{pair}")
                  for pair in range(2)}

            # ---- input DMA, need-ordered ----
            for proj, wT in (("v", wvT), ("q", wqT), ("k", wkT)):
                nc.sync.dma_start(
                    w_sb[proj][:], wT.rearrange("(dc di) c -> di dc c", di=P))
            for tt in range(NQT):
                ts = slice(tt * 512, (tt + 1) * 512)
                for dc in range(DC):
                    nc.sync.dma_start(xt_sb[:, dc, ts], xT3[:, dc, ts])
                nc.sync.dma_start(cos_sb[:, ts], cosP[:, ts])
                nc.sync.dma_start(sin_sb[:, ts], sinP[:, ts])
                if tt == 0:
                    nc.sync.dma_start(wo_sb[:], woT3[:])

            # constants: ones column of vp; triangle mask
            nc.gpsimd.memset(vp[:, :, :, HD:HD + 1], 1.0)
            nc.gpsimd.memset(tri[:], 1.0)
            nc.gpsimd.affine_select(
                tri[:], tri[:], [[1, P]], mybir.AluOpType.is_ge, 0.0,
                base=0, channel_multiplier=-1)

            # ---- V projection (token-major, xt stationary) ----
            with tc.tile_pool(name="psV", bufs=2, space="PSUM") as psV:
                for tbp in range(NTB // 2):
                    ps = psV.tile([P, 2, NH, HD], F32, tag="v")
                    for half in range(2):
                        tb = 2 * tbp + half
                        for dc in range(DC):
                            nc.tensor.matmul(
                                ps[:, half], xt_sb[:, dc, tb * P:(tb + 1) * P],
                                w_sb["v"][:, dc, :],
                                start=(dc == 0), stop=(dc == DC - 1))
                    nc.vector.tensor_copy(
                        vp[:, 2 * tbp:2 * tbp + 2, :, 0:HD], ps[:])

            # ---- fused per-tt pipeline ----
            with (
                tc.tile_pool(name="psSC", bufs=2, space="PSUM") as psSC,
                tc.tile_pool(name="psAV", bufs=2, space="PSUM") as psAV,
                tc.tile_pool(name="psOP", bufs=2, space="PSUM") as psOP,
            ):
                for tt in range(NQT):
                    ts = slice(tt * 512, (tt + 1) * 512)
                    # Q/K projection + rope for this 512-token chunk
                    for proj, pair in (("q", 0), ("k", 0), ("q", 1), ("k", 1)):
                        psq = psQK.tile([P, 512], F32, tag="qk")
                        for dc in range(DC):
                            nc.tensor.matmul(
                                psq[:],
                                w_sb[proj][:, dc, pair * P:(pair + 1) * P],
                                xt_sb[:, dc, ts],
                                start=(dc == 0), stop=(dc == DC - 1))
                        qb = shp.tile([P, 512], BF16, tag="qb")
                        nc.vector.tensor_copy(qb[:], psq[:])
                        sh = shp.tile([P, 512], BF16, tag="sh")
                        nc.vector.stream_shuffle(sh[:], qb[:], XOR1)
                        sh2 = shp.tile([P, 512], BF16, tag="sh2")
                        nc.gpsimd.tensor_mul(sh2[:], sh[:], sin_sb[:, ts])
                        dst = qk[(proj, pair)]
                        nc.vector.tensor_mul(dst[:, ts], qb[:], cos_sb[:, ts])
                        nc.vector.tensor_add(dst[:, ts], dst[:, ts], sh2[:])

                    # attention stage qt = tt (both pairs)
                    qt = tt
                    nkb = 4 * qt + 4
                    for pair in range(2):
                        qtile = qk[("q", pair)]
                        ktile = qk[("k", pair)]
                        avs = [psAV.tile([HD + 1, 512], F32, tag="av",
                                         name=f"av{pair}{qt}o{o}")
                               for o in range(2)]
                        pend = None
                        for kb in range(nkb):
                            off = max(0, (kb - 4 * qt) * P)
                            w_ = 512 - off
                            ets = []
                            for o in range(2):
                                hs = slice(64 * o, 64 * o + 64)
                                sc = psSC.tile([P, 512], F32, tag="sc")
                                nc.tensor.matmul(
                                    sc[:, 0:w_],
                                    ktile[hs, kb * P:(kb + 1) * P],
                                    qtile[hs, qt * 512 + off:(qt + 1) * 512],
                                    start=True, stop=True)
                                et = ex.tile([P, 512], BF16, tag="et")
                                nc.scalar.activation(
                                    et[:, 0:w_], sc[:, 0:w_], EXP, scale=0.125)
                                if kb >= 4 * qt:
                                    nc.vector.tensor_mul(
                                        et[:, 0:P], et[:, 0:P], tri[:])
                                ets.append(et)
                            if pend is not None:
                                pkb, poff, pw, pets = pend
                                for o in range(2):
                                    nc.tensor.matmul(
                                        avs[o][:, poff:512],
                                        vp[:, pkb, 2 * pair + o, :],
                                        pets[o][:, 0:pw],
                                        start=(pkb == 0),
                                        stop=(pkb == nkb - 1),
                                        skip_group_check=True)
                            pend = (kb, off, w_, ets)
                        pkb, poff, pw, pets = pend
                        for o in range(2):
                            nc.tensor.matmul(
                                avs[o][:, poff:512],
                                vp[:, pkb, 2 * pair + o, :],
                                pets[o][:, 0:pw],
                                start=(pkb == 0), stop=(pkb == nkb - 1),
                                skip_group_check=True)
                        # normalize: yt = av / den, 1/den = exp(-ln(den)) on ACT
                        for o in range(2):
                            lnd = nrm.tile([1, 512], F32, tag="lnd")
                            nc.scalar.activation(
                                lnd[:], avs[o][64:65, :], LN)
                            rec = nrm.tile([1, 512], F32, tag="rec")
                            nc.scalar.activation(
                                rec[:], lnd[:], EXP, scale=-1.0)
                            rb = nrm.tile([64, 512], F32, tag="rb")
                            nc.gpsimd.partition_broadcast(rb[:], rec[:])
                            nc.vector.tensor_mul(
                                yt[pair][64 * o:64 * o + 64, ts],
                                avs[o][0:64, :], rb[:])

                    # out-projection for this qt's 4 token blocks
                    for tb in range(4 * qt, 4 * qt + 4):
                        tbs = slice(tb * P, (tb + 1) * P)
                        for oc in range(2):
                            pt = psOP.tile([P, 512], F32, tag="op")
                            for cp in range(2):
                                nc.tensor.matmul(
                                    pt[:], yt[cp][:, tbs],
                                    wo_sb[:, cp, oc * 512:(oc + 1) * 512],
                                    start=(cp == 0), stop=(cp == 1))
                            ot = ob.tile([P, 512], F32, tag="ot")
                            nc.scalar.copy(ot[:], pt[:])
                            nc.sync.dma_start(
                                out[tbs, oc * 512:(oc + 1) * 512], ot[:])

    nc.finalize()
    return nc


def _prep_core_inputs(x, pos, Wq, Wk, Wv, Wo):
    """Per-core input dicts (host-side sharding + layout prep)."""
    inv_freq = THETA ** (-np.arange(0, HD, 2, dtype=np.float32) / HD)
    ang = pos.astype(np.float32)[:, None] * inv_freq[None, :]   # (S, 32)
    cos = np.cos(ang).astype(np.float32)                        # (S, 32)
    sin = np.sin(ang).astype(np.float32)
    p = np.arange(P)
    pairidx = (p % HD) // 2
    cosP = np.ascontiguousarray(cos[:, pairidx].T)              # (128, S)
    sgn = np.where(p % 2 == 0, -1.0, 1.0).astype(np.float32)
    sinP = np.ascontiguousarray(sin[:, pairidx].T * sgn[:, None])

    bf = ml_dtypes.bfloat16
    cosPb = cosP.astype(bf)
    sinPb = sinP.astype(bf)
    xTs = [np.ascontiguousarray(x[b].T).astype(bf) for b in range(B)]  # (D, S)
    maps = []
    for c in range(NCORES):
        b, g = divmod(c, NH)
        cs = slice(C * g, C * (g + 1))
        maps.append({
            "xT": xTs[b],
            "wqT": np.ascontiguousarray(Wq[cs, :].T).astype(bf),
            "wkT": np.ascontiguousarray(Wk[cs, :].T).astype(bf),
            "wvT": np.ascontiguousarray(Wv[cs, :].T).astype(bf),
            "woT": np.ascontiguousarray(Wo[:, cs].T).astype(bf),
            "cosP": cosPb,
            "sinP": sinPb,
        })
    return maps


def kernel(in_features, token_positions, Wq, Wk, Wv, Wo):
    global _NC_CACHE, LAST_RESULTS
    x = np.asarray(in_features, dtype=np.float32)
    pos = np.asarray(token_positions)
    Wq = np.asarray(Wq, dtype=np.float32)
    Wk = np.asarray(Wk, dtype=np.float32)
    Wv = np.asarray(Wv, dtype=np.float32)
    Wo = np.asarray(Wo, dtype=np.float32)

    if _NC_CACHE is None:
        _NC_CACHE = _build()
    maps = _prep_core_inputs(x, pos, Wq, Wk, Wv, Wo)
    res = run_bass_kernel_spmd(_NC_CACHE, maps, core_ids=list(range(NCORES)))
    LAST_RESULTS = res
    parts = [r["out"] for r in res.results]
    outb = [parts[4 * b] + parts[4 * b + 1] + parts[4 * b + 2] + parts[4 * b + 3]
            for b in range(B)]
    return np.stack(outb).astype(np.float32)


if __name__ == "__main__":
    rng = np.random.default_rng(0)
    x = rng.standard_normal((B, S, D), dtype=np.float32)
    o = kernel(x, np.arange(S, dtype=np.int32),
               *(rng.standard_normal((D, D), dtype=np.float32) / 32
                 for _ in range(4)))
    print(o.shape, o.dtype)


# revision 22
# speedup vs baseline: 1.0864x; 1.0864x over previous
"""MultiHeadAttention with RoPE on 8 Trainium2 NeuronCores.

Sharding: batch (2) x head-group (4 heads each) -> 8 cores. Each core
computes q/k/v projections for its 4 heads of one batch element, causal
attention, and a partial output projection (row-shard of Wo). The host
sums the 4 partial outputs per batch element (the "all-reduce").

v3: single fused pipeline to keep the PE dense (HAM warm) end to end.
  - Input DMA split per 512-token chunk (tt-major) so projections start
    ~2us in instead of after the full 7MiB load.
  - Per-tt stage: Q/K proj + rope for chunk tt, then attention stage
    qt=tt (scores/exp/AV for all kb), then out-projection for its 4
    token blocks.  V projection (token-major, xt stationary) runs first,
    double-buffered in its own scoped PSUM pool.
  - softmax denominators: reciprocal_approx_fast on the ones-row (was
    exact DVE reciprocal on a 1-partition AP = 3.4us each), then gpsimd
    partition_broadcast + DVE multiply.
  - rope done in bf16 off a psum->sbuf cast (2x DVE rate, early psum
    release); sin-mul on gpsimd.
  - causal diagonal mask via DVE multiply with a precomputed triangle
    tile (affine_select on gpsimd was sem-overhead heavy).
  - out-projection split per 512-col half (1-bank psum tiles), psum->
    sbuf eviction on ACT, store fp32.

PSUM budget: V(2, scoped) -> QK(2) + SC(2) + AV(2) + OP(2) = 8 banks.
"""

import numpy as np
import ml_dtypes

import concourse.bacc as bacc
import concourse.mybir as mybir
import concourse.tile as tile
from concourse.bass_utils import run_bass_kernel_spmd

F32 = mybir.dt.float32
BF16 = mybir.dt.bfloat16
EXP = mybir.ActivationFunctionType.Exp
LN = mybir.ActivationFunctionType.Ln

B, S, D = 2, 2048, 1024
H, HD = 16, 64
THETA = 10000.0
NCORES = 8
NH = 4          # heads per core
C = NH * HD     # 256 channels per core
P = 128
DC = D // P     # 8 contraction chunks
NQT = S // 512  # 4 q-tiles
NTB = S // P    # 16 token blocks

_NC_CACHE = None
LAST_RESULTS = None


def _patch_act_tables(nc):
    """Make Ln/Exp/Copy resolve to one activation-table set.

    The table-load inserter assigns each activation the first set
    containing its function; Exp and Ln land in different sets, so
    alternating them thrashes ACT_TABLE_LOAD (~1.3us each).  Removing
    the three functions we use from every other set leaves exactly one
    choice and one load."""
    import os
    if os.environ.get("NO_ACT_PATCH"):
        return
    try:
        from concourse.hw_specs import get_activation_tables
        tabs = get_activation_tables(nc.m.arch)
    except Exception:
        return
    combo = None
    for name, fns in tabs.items():
        if EXP in fns and LN in fns:
            combo = name
            break
    if combo is None:
        return
    keep = {EXP, LN, mybir.ActivationFunctionType.Copy}
    for name, fns in tabs.items():
        if name != combo:
            fns -= keep


def _build():
    nc = bacc.Bacc(None)
    _patch_act_tables(nc)

    xT = nc.dram_tensor("xT", [D, S], BF16, kind="ExternalInput")
    wqT = nc.dram_tensor("wqT", [D, C], BF16, kind="ExternalInput")
    wkT = nc.dram_tensor("wkT", [D, C], BF16, kind="ExternalInput")
    wvT = nc.dram_tensor("wvT", [D, C], BF16, kind="ExternalInput")
    woT = nc.dram_tensor("woT", [C, D], BF16, kind="ExternalInput")
    cosP = nc.dram_tensor("cosP", [P, S], BF16, kind="ExternalInput")
    sinP = nc.dram_tensor("sinP", [P, S], BF16, kind="ExternalInput")
    out = nc.dram_tensor("out", [S, D], F32, kind="ExternalOutput")

    xT3 = xT.rearrange("(dc di) t -> di dc t", di=P)
    woT3 = woT.rearrange("(cp ci) o -> ci cp o", ci=P)

    XOR1 = [i ^ 1 for i in range(32)]

    with tile.TileContext(nc) as tc:
        with (
            tc.tile_pool(name="cn", bufs=1) as cn,
            tc.tile_pool(name="big", bufs=1) as big,
            tc.tile_pool(name="psQK", bufs=2, space="PSUM") as psQK,
            tc.tile_pool(name="shp", bufs=3) as shp,
            tc.tile_pool(name="ex", bufs=6) as ex,
            tc.tile_pool(name="nrm", bufs=4) as nrm,
            tc.tile_pool(name="ob", bufs=3) as ob,
        ):
            # ---- long-lived tiles ----
            cos_sb = cn.tile([P, S], BF16, tag="cos")
            sin_sb = cn.tile([P, S], BF16, tag="sin")
            w_sb = {}
            for proj in ("v", "q", "k"):
                w_sb[proj] = cn.tile([P, DC, C], BF16, tag=f"w{proj}",
                                     name=f"w{proj}")
            xt_sb = cn.tile([P, DC, S], BF16, tag="xt")
            wo_sb = cn.tile([P, 2, D], BF16, tag="wo")
            # V in token-major blocks: [tok, tb, head, hd | ones]
            vp = cn.tile([P, NTB, NH, HD + 1], BF16, tag="vp")
            # causal-mask poison operands: out = eyeneg.T @ tri2 writes
            # -1e30 where key > query (strict upper triangle in (k, q))
            eyeneg = cn.tile([P, P], BF16, tag="eyeneg")
            tri2 = cn.tile([P, P], BF16, tag="tri2")
            # softmax-denominator staging: the pair's two den rows live at
            # partitions 0 and 32 (SBUF APs must start at 0/32/64/96) so one
            # ACT ln + one ACT exp cover both heads
            den33 = cn.tile([33, 512], F32, tag="den33")
            lnd33 = cn.tile([33, 512], F32, tag="lnd33")
            rec33 = cn.tile([33, 512], F32, tag="rec33")

            qk = {}
            for proj in ("q", "k"):
                for pair in range(2):
                    qk[(proj, pair)] = big.tile(
                        [P, S], BF16, tag=f"{proj}{pair}", name=f"{proj}{pair}")
            yt = {pair: big.tile([P, S], BF16, tag=f"y{pair}", name=f"y{pair}")
                  for pair in range(2)}

            # ---- input DMA, need-ordered ----
            for proj, wT in (("v", wvT), ("q", wqT), ("k", wkT)):
                nc.sync.dma_start(
                    w_sb[proj][:], wT.rearrange("(dc di) c -> di dc c", di=P))
            for tt in range(NQT):
                ts = slice(tt * 512, (tt + 1) * 512)
                for dc in range(DC):
                    nc.sync.dma_start(xt_sb[:, dc, ts], xT3[:, dc, ts])
                nc.sync.dma_start(cos_sb[:, ts], cosP[:, ts])
                nc.sync.dma_start(sin_sb[:, ts], sinP[:, ts])
                if tt == 0:
                    nc.sync.dma_start(wo_sb[:], woT3[:])

            # constants: ones column of vp; mask-poison operands
            nc.gpsimd.memset(vp[:, :, :, HD:HD + 1], 1.0)
            # walrus only implements is_ge for affine_select; build the
            # diagonal as the intersection of two is_ge half-planes
            nc.gpsimd.memset(eyeneg[:], -1e30)
            nc.gpsimd.affine_select(
                eyeneg[:], eyeneg[:], [[-1, P]], mybir.AluOpType.is_ge, 0.0,
                base=0, channel_multiplier=1)
            nc.gpsimd.affine_select(
                eyeneg[:], eyeneg[:], [[1, P]], mybir.AluOpType.is_ge, 0.0,
                base=0, channel_multiplier=-1)
            nc.gpsimd.memset(tri2[:], 1.0)
            nc.gpsimd.affine_select(
                tri2[:], tri2[:], [[-1, P]], mybir.AluOpType.is_ge, 0.0,
                base=-1, channel_multiplier=1)
            nc.gpsimd.memset(den33[:], 1.0)

            # ---- V projection (token-major, xt stationary) ----
            with tc.tile_pool(name="psV", bufs=2, space="PSUM") as psV:
                for tbp in range(NTB // 2):
                    ps = psV.tile([P, 2, NH, HD], F32, tag="v")
                    for half in range(2):
                        tb = 2 * tbp + half
                        for dc in range(DC):
                            nc.tensor.matmul(
                                ps[:, half], xt_sb[:, dc, tb * P:(tb + 1) * P],
                                w_sb["v"][:, dc, :],
                                start=(dc == 0), stop=(dc == DC - 1))
                    nc.vector.tensor_copy(
                        vp[:, 2 * tbp:2 * tbp + 2, :, 0:HD], ps[:])

            # ---- fused per-tt pipeline ----
            with (
                tc.tile_pool(name="psSC", bufs=1, space="PSUM") as psSC,
                tc.tile_pool(name="psAV", bufs=2, space="PSUM") as psAV,
                tc.tile_pool(name="psOP", bufs=2, space="PSUM") as psOP,
            ):
                for tt in range(NQT):
                    ts = slice(tt * 512, (tt + 1) * 512)
                    # Q/K projection + rope for this 512-token chunk
                    for proj, pair in (("q", 0), ("k", 0), ("q", 1), ("k", 1)):
                        psq = psQK.tile([P, 512], F32, tag="qk")
                        for dc in range(DC):
                            nc.tensor.matmul(
                                psq[:],
                                w_sb[proj][:, dc, pair * P:(pair + 1) * P],
                                xt_sb[:, dc, ts],
                                start=(dc == 0), stop=(dc == DC - 1))
                        qb = shp.tile([P, 512], BF16, tag="qb")
                        nc.vector.tensor_copy(qb[:], psq[:])
                        sh = shp.tile([P, 512], BF16, tag="sh")
                        nc.vector.stream_shuffle(sh[:], qb[:], XOR1)
                        sh2 = shp.tile([P, 512], BF16, tag="sh2")
                        nc.gpsimd.tensor_mul(sh2[:], sh[:], sin_sb[:, ts])
                        dst = qk[(proj, pair)]
                        nc.vector.tensor_mul(dst[:, ts], qb[:], cos_sb[:, ts])
                        nc.vector.tensor_add(dst[:, ts], dst[:, ts], sh2[:])

                    # attention stage qt = tt (both pairs)
                    qt = tt
                    nkb = 4 * qt + 4
                    for pair in range(2):
                        qtile = qk[("q", pair)]
                        ktile = qk[("k", pair)]
                        avs = [psAV.tile([HD + 1, 512], F32, tag="av",
                                         name=f"av{pair}{qt}o{o}")
                               for o in range(2)]
                        def emit_av(pend):
                            pkb, poff, pw, et = pend
                            for o in range(2):
                                nc.tensor.matmul(
                                    avs[o][:, poff:512],
                                    vp[:, pkb, 2 * pair + o, :],
                                    et[:, 512 * o:512 * o + pw],
                                    start=(pkb == 0), stop=(pkb == nkb - 1),
                                    skip_group_check=True)

                        pend = None
                        for kb in range(nkb):
                            off = max(0, (kb - 4 * qt) * P)
                            w_ = 512 - off
                            diag = kb >= 4 * qt
                            # both heads side by side in one 2-bank tile so
                            # the exp can be a single ACT instruction
                            sc = psSC.tile([P, 1024], F32, tag="sc")
                            for o in range(2):
                                hs = slice(64 * o, 64 * o + 64)
                                base = 512 * o
                                klhs = ktile[hs, kb * P:(kb + 1) * P]
                                qs0 = qt * 512 + off
                                if diag:
                                    # poison the diagonal 128-col block with
                                    # -1e30 above the diagonal; score matmul
                                    # accumulates on top (has_written trick),
                                    # split at col 128 so each piece is
                                    # uniformly accumulate/overwrite.
                                    nc.tensor.matmul(
                                        sc[:, base:base + P], eyeneg[:],
                                        tri2[:], start=True, stop=True,
                                        skip_group_check=True)
                                    nc.tensor.matmul(
                                        sc[:, base:base + P], klhs,
                                        qtile[hs, qs0:qs0 + P],
                                        start=False, stop=True,
                                        skip_group_check=True)
                                    if w_ > P:
                                        nc.tensor.matmul(
                                            sc[:, base + P:base + w_], klhs,
                                            qtile[hs, qs0 + P:(qt + 1) * 512],
                                            start=False, stop=True,
                                            skip_group_check=True)
                                else:
                                    nc.tensor.matmul(
                                        sc[:, base:base + w_], klhs,
                                        qtile[hs, qs0:(qt + 1) * 512],
                                        start=True, stop=True,
                                        skip_group_check=True)
                            et = ex.tile([P, 1024], BF16, tag="et")
                            if w_ == 512:
                                nc.scalar.activation(
                                    et[:], sc[:], EXP, scale=0.125)
                            else:
                                for o in range(2):
                                    nc.scalar.activation(
                                        et[:, 512 * o:512 * o + w_],
                                        sc[:, 512 * o:512 * o + w_],
                                        EXP, scale=0.125)
                            if pend is not None:
                                emit_av(pend)
                            pend = (kb, off, w_, et)
                        emit_av(pend)
                        # normalize: yt = av * exp(-ln(den)); reciprocal via
                        # ACT ln/exp batched over the pair's two heads
                        for o in range(2):
                            nc.vector.tensor_copy(
                                den33[32 * o:32 * o + 1, :], avs[o][64:65, :])
                        nc.scalar.activation(lnd33[:], den33[:], LN)
                        nc.scalar.activation(rec33[:], lnd33[:], EXP,
                                             scale=-1.0)
                        # partition_broadcast ignores the AP base partition on
                        # HW (always reads physical partition 0): relocate
                        # head 1's row to its own tile first.
                        recb = nrm.tile([1, 512], F32, tag="recb")
                        nc.vector.tensor_copy(recb[:], rec33[32:33, :])
                        for o in range(2):
                            rb = nrm.tile([64, 512], F32, tag="rb")
                            nc.gpsimd.partition_broadcast(
                                rb[:], rec33[0:1, :] if o == 0 else recb[:])
                            nc.vector.tensor_mul(
                                yt[pair][64 * o:64 * o + 64, ts],
                                avs[o][0:64, :], rb[:])

                    # out-projection for this qt's 4 token blocks
                    for tb in range(4 * qt, 4 * qt + 4):
                        tbs = slice(tb * P, (tb + 1) * P)
                        for oc in range(2):
                            pt = psOP.tile([P, 512], F32, tag="op")
                            for cp in range(2):
                                nc.tensor.matmul(
                                    pt[:], yt[cp][:, tbs],
                                    wo_sb[:, cp, oc * 512:(oc + 1) * 512],
                                    start=(cp == 0), stop=(cp == 1))
                            ot = ob.tile([P, 512], F32, tag="ot")
                            nc.vector.tensor_copy(ot[:], pt[:])
                            nc.sync.dma_start(
                                out[tbs, oc * 512:(oc + 1) * 512], ot[:])

    nc.finalize()
    return nc


def _prep_core_inputs(x, pos, Wq, Wk, Wv, Wo):
    """Per-core input dicts (host-side sharding + layout prep)."""
    inv_freq = THETA ** (-np.arange(0, HD, 2, dtype=np.float32) / HD)
    ang = pos.astype(np.float32)[:, None] * inv_freq[None, :]   # (S, 32)
    cos = np.cos(ang).astype(np.float32)                        # (S, 32)
    sin = np.sin(ang).astype(np.float32)
    p = np.arange(P)
    pairidx = (p % HD) // 2
    cosP = np.ascontiguousarray(cos[:, pairidx].T)              # (128, S)
    sgn = np.where(p % 2 == 0, -1.0, 1.0).astype(np.float32)
    sinP = np.ascontiguousarray(sin[:, pairidx].T * sgn[:, None])

    bf = ml_dtypes.bfloat16
    cosPb = cosP.astype(bf)
    sinPb = sinP.astype(bf)
    xTs = [np.ascontiguousarray(x[b].T).astype(bf) for b in range(B)]  # (D, S)
    maps = []
    for c in range(NCORES):
        b, g = divmod(c, NH)
        cs = slice(C * g, C * (g + 1))
        maps.append({
            "xT": xTs[b],
            "wqT": np.ascontiguousarray(Wq[cs, :].T).astype(bf),
            "wkT": np.ascontiguousarray(Wk[cs, :].T).astype(bf),
            "wvT": np.ascontiguousarray(Wv[cs, :].T).astype(bf),
            "woT": np.ascontiguousarray(Wo[:, cs].T).astype(bf),
            "cosP": cosPb,
            "sinP": sinPb,
        })
    return maps


def kernel(in_features, token_positions, Wq, Wk, Wv, Wo):
    global _NC_CACHE, LAST_RESULTS
    x = np.asarray(in_features, dtype=np.float32)
    pos = np.asarray(token_positions)
    Wq = np.asarray(Wq, dtype=np.float32)
    Wk = np.asarray(Wk, dtype=np.float32)
    Wv = np.asarray(Wv, dtype=np.float32)
    Wo = np.asarray(Wo, dtype=np.float32)

    if _NC_CACHE is None:
        _NC_CACHE = _build()
    maps = _prep_core_inputs(x, pos, Wq, Wk, Wv, Wo)
    res = run_bass_kernel_spmd(_NC_CACHE, maps, core_ids=list(range(NCORES)))
    LAST_RESULTS = res
    parts = [r["out"] for r in res.results]
    outb = [parts[4 * b] + parts[4 * b + 1] + parts[4 * b + 2] + parts[4 * b + 3]
            for b in range(B)]
    return np.stack(outb).astype(np.float32)


if __name__ == "__main__":
    rng = np.random.default_rng(0)
    x = rng.standard_normal((B, S, D), dtype=np.float32)
    o = kernel(x, np.arange(S, dtype=np.int32),
               *(rng.standard_normal((D, D), dtype=np.float32) / 32
                 for _ in range(4)))
    print(o.shape, o.dtype)


# revision 27
# speedup vs baseline: 1.3110x; 1.2068x over previous
"""MultiHeadAttention with RoPE on 8 Trainium2 NeuronCores.

Sharding: batch (2) x head-group (4 heads each) -> 8 cores. Each core
computes q/k/v projections for its 4 heads of one batch element, causal
attention, and a partial output projection (row-shard of Wo). The host
sums the 4 partial outputs per batch element (the "all-reduce").

v3: single fused pipeline to keep the PE dense (HAM warm) end to end.
  - Input DMA split per 512-token chunk (tt-major) so projections start
    ~2us in instead of after the full 7MiB load.
  - Per-tt stage: Q/K proj + rope for chunk tt, then attention stage
    qt=tt (scores/exp/AV for all kb), then out-projection for its 4
    token blocks.  V projection (token-major, xt stationary) runs first,
    double-buffered in its own scoped PSUM pool.
  - softmax denominators: reciprocal_approx_fast on the ones-row (was
    exact DVE reciprocal on a 1-partition AP = 3.4us each), then gpsimd
    partition_broadcast + DVE multiply.
  - rope done in bf16 off a psum->sbuf cast (2x DVE rate, early psum
    release); sin-mul on gpsimd.
  - causal diagonal mask via DVE multiply with a precomputed triangle
    tile (affine_select on gpsimd was sem-overhead heavy).
  - out-projection split per 512-col half (1-bank psum tiles), psum->
    sbuf eviction on ACT, store fp32.

PSUM budget: V(2, scoped) -> QK(2) + SC(2) + AV(2) + OP(2) = 8 banks.
"""

import numpy as np
import ml_dtypes

import concourse.bacc as bacc
import concourse.mybir as mybir
import concourse.tile as tile
from concourse.bass_utils import run_bass_kernel_spmd

F32 = mybir.dt.float32
BF16 = mybir.dt.bfloat16
EXP = mybir.ActivationFunctionType.Exp
LN = mybir.ActivationFunctionType.Ln

B, S, D = 2, 2048, 1024
H, HD = 16, 64
THETA = 10000.0
NCORES = 8
NH = 4          # heads per core
C = NH * HD     # 256 channels per core
P = 128
DC = D // P     # 8 contraction chunks
NQT = S // 512  # 4 q-tiles
NTB = S // P    # 16 token blocks

_NC_CACHE = None
LAST_RESULTS = None


def _patch_act_tables(nc):
    """Make Ln/Exp/Copy resolve to one activation-table set.

    The table-load inserter assigns each activation the first set
    containing its function; Exp and Ln land in different sets, so
    alternating them thrashes ACT_TABLE_LOAD (~1.3us each).  Removing
    the three functions we use from every other set leaves exactly one
    choice and one load."""
    import os
    if os.environ.get("NO_ACT_PATCH"):
        return
    try:
        from concourse.hw_specs import get_activation_tables
        tabs = get_activation_tables(nc.m.arch)
    except Exception:
        return
    combo = None
    for name, fns in tabs.items():
        if EXP in fns and LN in fns:
            combo = name
            break
    if combo is None:
        return
    keep = {EXP, LN, mybir.ActivationFunctionType.Copy}
    for name, fns in tabs.items():
        if name != combo:
            fns -= keep


def _build():
    nc = bacc.Bacc(None)
    _patch_act_tables(nc)

    xT = nc.dram_tensor("xT", [D, S], BF16, kind="ExternalInput")
    wqT = nc.dram_tensor("wqT", [D, C], BF16, kind="ExternalInput")
    wkT = nc.dram_tensor("wkT", [D, C], BF16, kind="ExternalInput")
    wvT = nc.dram_tensor("wvT", [D, C], BF16, kind="ExternalInput")
    woT = nc.dram_tensor("woT", [C, D], BF16, kind="ExternalInput")
    cosP = nc.dram_tensor("cosP", [P, S], BF16, kind="ExternalInput")
    sinP = nc.dram_tensor("sinP", [P, S], BF16, kind="ExternalInput")
    out = nc.dram_tensor("out", [S, D], F32, kind="ExternalOutput")

    xT3 = xT.rearrange("(dc di) t -> di dc t", di=P)
    woT3 = woT.rearrange("(cp ci) o -> ci cp o", ci=P)

    XOR1 = [i ^ 1 for i in range(32)]

    with tile.TileContext(nc) as tc:
        with (
            tc.tile_pool(name="cn", bufs=1) as cn,
            tc.tile_pool(name="big", bufs=1) as big,
            tc.tile_pool(name="psQK", bufs=2, space="PSUM") as psQK,
            tc.tile_pool(name="shp", bufs=3) as shp,
            tc.tile_pool(name="ex", bufs=6) as ex,
            tc.tile_pool(name="nrm", bufs=4) as nrm,
            tc.tile_pool(name="ob", bufs=3) as ob,
        ):
            # ---- long-lived tiles ----
            cos_sb = cn.tile([P, S], BF16, tag="cos")
            sin_sb = cn.tile([P, S], BF16, tag="sin")
            w_sb = {}
            for proj in ("v", "q", "k"):
                w_sb[proj] = cn.tile([P, DC, C], BF16, tag=f"w{proj}",
                                     name=f"w{proj}")
            xt_sb = cn.tile([P, DC, S], BF16, tag="xt")
            wo_sb = cn.tile([P, 2, D], BF16, tag="wo")
            # V in token-major blocks: [tok, tb, head, hd | ones]
            vp = cn.tile([P, NTB, NH, HD + 1], BF16, tag="vp")
            # causal-mask poison operands: out = eyeneg.T @ tri2 writes
            # -1e30 where key > query (strict upper triangle in (k, q))
            eyeneg = cn.tile([P, P], BF16, tag="eyeneg")
            tri2 = cn.tile([P, P], BF16, tag="tri2")
            # softmax-denominator staging: the pair's two den rows live at
            # partitions 0 and 32 (SBUF APs must start at 0/32/64/96) so one
            # ACT ln + one ACT exp cover both heads
            den33 = cn.tile([33, 512], F32, tag="den33")
            lnd33 = cn.tile([33, 512], F32, tag="lnd33")
            rec33 = cn.tile([33, 512], F32, tag="rec33")

            qk = {}
            for proj in ("q", "k"):
                for pair in range(2):
                    qk[(proj, pair)] = big.tile(
                        [P, S], BF16, tag=f"{proj}{pair}", name=f"{proj}{pair}")
            yt = {pair: big.tile([P, S], BF16, tag=f"y{pair}", name=f"y{pair}")
                  for pair in range(2)}

            # ---- input DMA, need-ordered ----
            for proj, wT in (("v", wvT), ("q", wqT), ("k", wkT)):
                nc.sync.dma_start(
                    w_sb[proj][:], wT.rearrange("(dc di) c -> di dc c", di=P))
            nc.sync.dma_start(cos_sb[:], cosP[:])
            nc.sync.dma_start(sin_sb[:], sinP[:])
            for tt in range(NQT):
                ts = slice(tt * 512, (tt + 1) * 512)
                nc.sync.dma_start(xt_sb[:, :, ts], xT3[:, :, ts])
                if tt == 0:
                    nc.sync.dma_start(wo_sb[:], woT3[:])

            # constants: ones column of vp; mask-poison operands
            nc.gpsimd.memset(vp[:, :, :, HD:HD + 1], 1.0)
            # walrus only implements is_ge for affine_select; build the
            # diagonal as the intersection of two is_ge half-planes
            nc.gpsimd.memset(eyeneg[:], -1e30)
            nc.gpsimd.affine_select(
                eyeneg[:], eyeneg[:], [[-1, P]], mybir.AluOpType.is_ge, 0.0,
                base=0, channel_multiplier=1)
            nc.gpsimd.affine_select(
                eyeneg[:], eyeneg[:], [[1, P]], mybir.AluOpType.is_ge, 0.0,
                base=0, channel_multiplier=-1)
            nc.gpsimd.memset(tri2[:], 1.0)
            nc.gpsimd.affine_select(
                tri2[:], tri2[:], [[-1, P]], mybir.AluOpType.is_ge, 0.0,
                base=-1, channel_multiplier=1)
            nc.gpsimd.memset(den33[:], 1.0)

            # ---- V projection (token-major, xt stationary) ----
            with tc.tile_pool(name="psV", bufs=2, space="PSUM") as psV:
                for tbp in range(NTB // 2):
                    ps = psV.tile([P, 2, NH, HD], F32, tag="v")
                    for half in range(2):
                        tb = 2 * tbp + half
                        for dc in range(DC):
                            nc.tensor.matmul(
                                ps[:, half], xt_sb[:, dc, tb * P:(tb + 1) * P],
                                w_sb["v"][:, dc, :],
                                start=(dc == 0), stop=(dc == DC - 1))
                    nc.vector.tensor_copy(
                        vp[:, 2 * tbp:2 * tbp + 2, :, 0:HD], ps[:])

            # ---- fused per-tt pipeline ----
            with (
                tc.tile_pool(name="psSC", bufs=1, space="PSUM") as psSC,
                tc.tile_pool(name="psAV", bufs=2, space="PSUM") as psAV,
                tc.tile_pool(name="psOP", bufs=2, space="PSUM") as psOP,
            ):
                opq = []

                def outproj_piece(tb):
                    tbs = slice(tb * P, (tb + 1) * P)
                    for oc in range(2):
                        pt = psOP.tile([P, 512], F32, tag="op",
                                       name=f"op{tb}_{oc}")
                        for cp in range(2):
                            nc.tensor.matmul(
                                pt[:], yt[cp][:, tbs],
                                wo_sb[:, cp, oc * 512:(oc + 1) * 512],
                                start=(cp == 0), stop=(cp == 1))
                        ot = ob.tile([P, 512], F32, tag="ot",
                                     name=f"ot{tb}_{oc}")
                        nc.vector.tensor_copy(ot[:], pt[:])
                        nc.sync.dma_start(
                            out[tbs, oc * 512:(oc + 1) * 512], ot[:])

                for tt in range(NQT):
                    ts = slice(tt * 512, (tt + 1) * 512)
                    # Q/K projection + rope for this 512-token chunk
                    for proj, pair in (("q", 0), ("k", 0), ("q", 1), ("k", 1)):
                        psq = psQK.tile([P, 512], F32, tag="qk")
                        for dc in range(DC):
                            nc.tensor.matmul(
                                psq[:],
                                w_sb[proj][:, dc, pair * P:(pair + 1) * P],
                                xt_sb[:, dc, ts],
                                start=(dc == 0), stop=(dc == DC - 1))
                        qb = shp.tile([P, 512], BF16, tag="qb")
                        nc.vector.tensor_copy(qb[:], psq[:])
                        sh = shp.tile([P, 512], BF16, tag="sh")
                        nc.vector.stream_shuffle(sh[:], qb[:], XOR1)
                        # all rope muls on DVE: gpsimd must stay on a single
                        # ucode library (partition_broadcast) or it thrashes
                        # ~7us per library swap
                        sh2 = shp.tile([P, 512], BF16, tag="sh2")
                        nc.vector.tensor_mul(sh2[:], sh[:], sin_sb[:, ts])
                        dst = qk[(proj, pair)]
                        nc.vector.tensor_mul(dst[:, ts], qb[:], cos_sb[:, ts])
                        nc.vector.tensor_add(dst[:, ts], dst[:, ts], sh2[:])

                    # attention stage qt = tt (both pairs)
                    qt = tt
                    nkb = 4 * qt + 4
                    for pair in range(2):
                        qtile = qk[("q", pair)]
                        ktile = qk[("k", pair)]
                        avs = [psAV.tile([HD + 1, 512], F32, tag="av",
                                         name=f"av{pair}{qt}o{o}")
                               for o in range(2)]
                        def emit_av(pend):
                            pkb, poff, pw, et = pend
                            for o in range(2):
                                nc.tensor.matmul(
                                    avs[o][:, poff:512],
                                    vp[:, pkb, 2 * pair + o, :],
                                    et[:, 512 * o:512 * o + pw],
                                    start=(pkb == 0), stop=(pkb == nkb - 1),
                                    skip_group_check=True)

                        pend = None
                        for kb in range(nkb):
                            off = max(0, (kb - 4 * qt) * P)
                            w_ = 512 - off
                            diag = kb >= 4 * qt
                            # both heads side by side in one 2-bank tile so
                            # the exp can be a single ACT instruction
                            sc = psSC.tile([P, 1024], F32, tag="sc")
                            for o in range(2):
                                hs = slice(64 * o, 64 * o + 64)
                                base = 512 * o
                                klhs = ktile[hs, kb * P:(kb + 1) * P]
                                qs0 = qt * 512 + off
                                if diag:
                                    # poison the diagonal 128-col block with
                                    # -1e30 above the diagonal; score matmul
                                    # accumulates on top (has_written trick),
                                    # split at col 128 so each piece is
                                    # uniformly accumulate/overwrite.
                                    nc.tensor.matmul(
                                        sc[:, base:base + P], eyeneg[:],
                                        tri2[:], start=True, stop=True,
                                        skip_group_check=True)
                                    nc.tensor.matmul(
                                        sc[:, base:base + P], klhs,
                                        qtile[hs, qs0:qs0 + P],
                                        start=False, stop=True,
                                        skip_group_check=True)
                                    if w_ > P:
                                        nc.tensor.matmul(
                                            sc[:, base + P:base + w_], klhs,
                                            qtile[hs, qs0 + P:(qt + 1) * 512],
                                            start=False, stop=True,
                                            skip_group_check=True)
                                else:
                                    nc.tensor.matmul(
                                        sc[:, base:base + w_], klhs,
                                        qtile[hs, qs0:(qt + 1) * 512],
                                        start=True, stop=True,
                                        skip_group_check=True)
                            et = ex.tile([P, 1024], BF16, tag="et")
                            if w_ == 512:
                                nc.scalar.activation(
                                    et[:], sc[:], EXP, scale=0.125)
                            else:
                                for o in range(2):
                                    nc.scalar.activation(
                                        et[:, 512 * o:512 * o + w_],
                                        sc[:, 512 * o:512 * o + w_],
                                        EXP, scale=0.125)
                            if pend is not None:
                                emit_av(pend)
                                if opq:
                                    outproj_piece(opq.pop(0))
                            pend = (kb, off, w_, et)
                        emit_av(pend)
                        # normalize: yt = av * exp(-ln(den)); reciprocal via
                        # ACT ln/exp batched over the pair's two heads
                        for o in range(2):
                            nc.vector.tensor_copy(
                                den33[32 * o:32 * o + 1, :], avs[o][64:65, :])
                        nc.scalar.activation(lnd33[:], den33[:], LN)
                        nc.scalar.activation(rec33[:], lnd33[:], EXP,
                                             scale=-1.0)
                        # partition_broadcast ignores the AP base partition on
                        # HW (always reads physical partition 0): relocate
                        # head 1's row to its own tile first.
                        recb = nrm.tile([1, 512], F32, tag="recb")
                        nc.vector.tensor_copy(recb[:], rec33[32:33, :])
                        for o in range(2):
                            rb = nrm.tile([64, 512], F32, tag="rb")
                            nc.gpsimd.partition_broadcast(
                                rb[:], rec33[0:1, :] if o == 0 else recb[:])
                            nc.vector.tensor_mul(
                                yt[pair][64 * o:64 * o + 64, ts],
                                avs[o][0:64, :], rb[:])

                    # queue this qt's out-projection; emitted interleaved
                    # into the next stage's unit stream (PE gap filler)
                    opq.extend(range(4 * qt, 4 * qt + 4))
                for tb in opq:
                    outproj_piece(tb)

    nc.finalize()
    return nc


def _prep_core_inputs(x, pos, Wq, Wk, Wv, Wo):
    """Per-core input dicts (host-side sharding + layout prep)."""
    inv_freq = THETA ** (-np.arange(0, HD, 2, dtype=np.float32) / HD)
    ang = pos.astype(np.float32)[:, None] * inv_freq[None, :]   # (S, 32)
    cos = np.cos(ang).astype(np.float32)                        # (S, 32)
    sin = np.sin(ang).astype(np.float32)
    p = np.arange(P)
    pairidx = (p % HD) // 2
    cosP = np.ascontiguousarray(cos[:, pairidx].T)              # (128, S)
    sgn = np.where(p % 2 == 0, -1.0, 1.0).astype(np.float32)
    sinP = np.ascontiguousarray(sin[:, pairidx].T * sgn[:, None])

    bf = ml_dtypes.bfloat16
    cosPb = cosP.astype(bf)
    sinPb = sinP.astype(bf)
    xTs = [np.ascontiguousarray(x[b].T).astype(bf) for b in range(B)]  # (D, S)
    maps = []
    for c in range(NCORES):
        b, g = divmod(c, NH)
        cs = slice(C * g, C * (g + 1))
        maps.append({
            "xT": xTs[b],
            "wqT": np.ascontiguousarray(Wq[cs, :].T).astype(bf),
            "wkT": np.ascontiguousarray(Wk[cs, :].T).astype(bf),
            "wvT": np.ascontiguousarray(Wv[cs, :].T).astype(bf),
            "woT": np.ascontiguousarray(Wo[:, cs].T).astype(bf),
            "cosP": cosPb,
            "sinP": sinPb,
        })
    return maps


def kernel(in_features, token_positions, Wq, Wk, Wv, Wo):
    global _NC_CACHE, LAST_RESULTS
    x = np.asarray(in_features, dtype=np.float32)
    pos = np.asarray(token_positions)
    Wq = np.asarray(Wq, dtype=np.float32)
    Wk = np.asarray(Wk, dtype=np.float32)
    Wv = np.asarray(Wv, dtype=np.float32)
    Wo = np.asarray(Wo, dtype=np.float32)

    if _NC_CACHE is None:
        _NC_CACHE = _build()
    maps = _prep_core_inputs(x, pos, Wq, Wk, Wv, Wo)
    res = run_bass_kernel_spmd(_NC_CACHE, maps, core_ids=list(range(NCORES)))
    LAST_RESULTS = res
    parts = [r["out"] for r in res.results]
    outb = [parts[4 * b] + parts[4 * b + 1] + parts[4 * b + 2] + parts[4 * b + 3]
            for b in range(B)]
    return np.stack(outb).astype(np.float32)


if __name__ == "__main__":
    rng = np.random.default_rng(0)
    x = rng.standard_normal((B, S, D), dtype=np.float32)
    o = kernel(x, np.arange(S, dtype=np.int32),
               *(rng.standard_normal((D, D), dtype=np.float32) / 32
                 for _ in range(4)))
    print(o.shape, o.dtype)


# revision 31
# speedup vs baseline: 1.5324x; 1.1689x over previous
"""MultiHeadAttention with RoPE on 8 Trainium2 NeuronCores.

Sharding: batch (2) x head-group (4 heads each) -> 8 cores. Each core
computes q/k/v projections for its 4 heads of one batch element, causal
attention, and a partial output projection (row-shard of Wo). The host
sums the 4 partial outputs per batch element (the "all-reduce").

v3: single fused pipeline to keep the PE dense (HAM warm) end to end.
  - Input DMA split per 512-token chunk (tt-major) so projections start
    ~2us in instead of after the full 7MiB load.
  - Per-tt stage: Q/K proj + rope for chunk tt, then attention stage
    qt=tt (scores/exp/AV for all kb), then out-projection for its 4
    token blocks.  V projection (token-major, xt stationary) runs first,
    double-buffered in its own scoped PSUM pool.
  - softmax denominators: reciprocal_approx_fast on the ones-row (was
    exact DVE reciprocal on a 1-partition AP = 3.4us each), then gpsimd
    partition_broadcast + DVE multiply.
  - rope done in bf16 off a psum->sbuf cast (2x DVE rate, early psum
    release); sin-mul on gpsimd.
  - causal diagonal mask via DVE multiply with a precomputed triangle
    tile (affine_select on gpsimd was sem-overhead heavy).
  - out-projection split per 512-col half (1-bank psum tiles), psum->
    sbuf eviction on ACT, store fp32.

PSUM budget: V(2, scoped) -> QK(2) + SC(2) + AV(2) + OP(2) = 8 banks.
"""

import numpy as np
import ml_dtypes

import concourse.bacc as bacc
import concourse.mybir as mybir
import concourse.tile as tile
from concourse.bass_utils import run_bass_kernel_spmd

F32 = mybir.dt.float32
BF16 = mybir.dt.bfloat16
EXP = mybir.ActivationFunctionType.Exp
LN = mybir.ActivationFunctionType.Ln

B, S, D = 2, 2048, 1024
H, HD = 16, 64
THETA = 10000.0
NCORES = 8
NH = 4          # heads per core
C = NH * HD     # 256 channels per core
P = 128
DC = D // P     # 8 contraction chunks
NQT = S // 512  # 4 q-tiles
NTB = S // P    # 16 token blocks

_NC_CACHE = None
LAST_RESULTS = None


def _patch_act_tables(nc):
    """Make Ln/Exp/Copy resolve to one activation-table set.

    The table-load inserter assigns each activation the first set
    containing its function; Exp and Ln land in different sets, so
    alternating them thrashes ACT_TABLE_LOAD (~1.3us each).  Removing
    the three functions we use from every other set leaves exactly one
    choice and one load."""
    import os
    if os.environ.get("NO_ACT_PATCH"):
        return
    try:
        from concourse.hw_specs import get_activation_tables
        tabs = get_activation_tables(nc.m.arch)
    except Exception:
        return
    combo = None
    for name, fns in tabs.items():
        if EXP in fns and LN in fns:
            combo = name
            break
    if combo is None:
        return
    keep = {EXP, LN, mybir.ActivationFunctionType.Copy}
    for name, fns in tabs.items():
        if name != combo:
            fns -= keep


def _build():
    nc = bacc.Bacc(None)
    _patch_act_tables(nc)

    xT = nc.dram_tensor("xT", [D, S], BF16, kind="ExternalInput")
    wqT = nc.dram_tensor("wqT", [D, C], BF16, kind="ExternalInput")
    wkT = nc.dram_tensor("wkT", [D, C], BF16, kind="ExternalInput")
    wvT = nc.dram_tensor("wvT", [D, C], BF16, kind="ExternalInput")
    woT = nc.dram_tensor("woT", [C, D], BF16, kind="ExternalInput")
    cosP = nc.dram_tensor("cosP", [P, S], BF16, kind="ExternalInput")
    sinP = nc.dram_tensor("sinP", [P, S], BF16, kind="ExternalInput")
    out = nc.dram_tensor("out", [S, D], F32, kind="ExternalOutput")

    xT3 = xT.rearrange("(dc di) t -> di dc t", di=P)
    woT3 = woT.rearrange("(cp ci) o -> ci cp o", ci=P)

    XOR1 = [i ^ 1 for i in range(32)]

    with tile.TileContext(nc) as tc:
        with (
            tc.tile_pool(name="cn", bufs=1) as cn,
            tc.tile_pool(name="big", bufs=1) as big,
            tc.tile_pool(name="psQK", bufs=1, space="PSUM") as psQK,
            tc.tile_pool(name="shp", bufs=3) as shp,
            tc.tile_pool(name="ex", bufs=6) as ex,
            tc.tile_pool(name="nrm", bufs=4) as nrm,
            tc.tile_pool(name="ob", bufs=3) as ob,
        ):
            # ---- long-lived tiles ----
            cos_sb = cn.tile([P, S], BF16, tag="cos")
            sin_sb = cn.tile([P, S], BF16, tag="sin")
            w_sb = {}
            for proj in ("v", "q", "k"):
                w_sb[proj] = cn.tile([P, DC, C], BF16, tag=f"w{proj}",
                                     name=f"w{proj}")
            xt_sb = cn.tile([P, DC, S], BF16, tag="xt")
            wo_sb = cn.tile([P, 2, D], BF16, tag="wo")
            # V in token-major blocks: [tok, tb, head, hd | ones]
            vp = cn.tile([P, NTB, NH, HD + 1], BF16, tag="vp")
            # causal-mask poison operands: out = eyeneg.T @ tri2 writes
            # -1e30 where key > query (strict upper triangle in (k, q))
            eyeneg = cn.tile([P, P], BF16, tag="eyeneg")
            tri2 = cn.tile([P, P], BF16, tag="tri2")
            # softmax-denominator staging: the pair's two den rows live at
            # partitions 0 and 32 (SBUF APs must start at 0/32/64/96) so one
            # ACT ln + one ACT exp cover both heads
            den33 = cn.tile([33, 512], F32, tag="den33")
            lnd33 = cn.tile([33, 512], F32, tag="lnd33")
            rec33 = cn.tile([33, 512], F32, tag="rec33")

            qk = {}
            for proj in ("q", "k"):
                for pair in range(2):
                    qk[(proj, pair)] = big.tile(
                        [P, S], BF16, tag=f"{proj}{pair}", name=f"{proj}{pair}")
            yt = {pair: big.tile([P, S], BF16, tag=f"y{pair}", name=f"y{pair}")
                  for pair in range(2)}

            # ---- input DMA, two staged groups ----
            # All queues share HBM bandwidth, so an unordered issue makes
            # everything finish together at ~19.5us.  Group A (weights +
            # rope tables + first token chunk) runs first; group B waits on
            # it, so stage-0 compute starts at ~10us instead of ~20us.
            from concourse.tile import add_dep_helper
            grpA = []
            for proj, wT in (("v", wvT), ("q", wqT), ("k", wkT)):
                grpA.append(nc.sync.dma_start(
                    w_sb[proj][:], wT.rearrange("(dc di) c -> di dc c", di=P)))
            grpA.append(nc.sync.dma_start(cos_sb[:], cosP[:]))
            grpA.append(nc.sync.dma_start(sin_sb[:], sinP[:]))
            grpA.append(nc.sync.dma_start(
                xt_sb[:, :, 0:512], xT3[:, :, 0:512]))
            grpB = []
            for tt in range(1, NQT):
                ts = slice(tt * 512, (tt + 1) * 512)
                grpB.append(nc.sync.dma_start(xt_sb[:, :, ts], xT3[:, :, ts]))
            grpB.append(nc.sync.dma_start(wo_sb[:], woT3[:]))
            for b in grpB:
                for a in grpA:
                    add_dep_helper(b.ins, a.ins, reason="input staging order")

            # constants: ones column of vp; mask-poison operands
            nc.gpsimd.memset(vp[:, :, :, HD:HD + 1], 1.0)
            # walrus only implements is_ge for affine_select; build the
            # diagonal as the intersection of two is_ge half-planes
            nc.gpsimd.memset(eyeneg[:], -1e30)
            nc.gpsimd.affine_select(
                eyeneg[:], eyeneg[:], [[-1, P]], mybir.AluOpType.is_ge, 0.0,
                base=0, channel_multiplier=1)
            nc.gpsimd.affine_select(
                eyeneg[:], eyeneg[:], [[1, P]], mybir.AluOpType.is_ge, 0.0,
                base=0, channel_multiplier=-1)
            nc.gpsimd.memset(tri2[:], 1.0)
            nc.gpsimd.affine_select(
                tri2[:], tri2[:], [[-1, P]], mybir.AluOpType.is_ge, 0.0,
                base=-1, channel_multiplier=1)
            nc.gpsimd.memset(den33[:], 1.0)

            # ---- V projection (token-major, xt stationary) ----
            with tc.tile_pool(name="psV", bufs=2, space="PSUM") as psV:
                for tbp in range(NTB // 2):
                    ps = psV.tile([P, 2, NH, HD], F32, tag="v")
                    for half in range(2):
                        tb = 2 * tbp + half
                        for dc in range(DC):
                            nc.tensor.matmul(
                                ps[:, half], xt_sb[:, dc, tb * P:(tb + 1) * P],
                                w_sb["v"][:, dc, :],
                                start=(dc == 0), stop=(dc == DC - 1))
                    nc.vector.tensor_copy(
                        vp[:, 2 * tbp:2 * tbp + 2, :, 0:HD], ps[:])

            # ---- fused per-tt pipeline ----
            with (
                tc.tile_pool(name="psSC", bufs=2, space="PSUM") as psSC,
                tc.tile_pool(name="psAV", bufs=2, space="PSUM") as psAV,
                tc.tile_pool(name="psOP", bufs=1, space="PSUM") as psOP,
            ):
                opq = []

                def outproj_piece(tb):
                    tbs = slice(tb * P, (tb + 1) * P)
                    for oc in range(2):
                        pt = psOP.tile([P, 512], F32, tag="op",
                                       name=f"op{tb}_{oc}")
                        for cp in range(2):
                            nc.tensor.matmul(
                                pt[:], yt[cp][:, tbs],
                                wo_sb[:, cp, oc * 512:(oc + 1) * 512],
                                start=(cp == 0), stop=(cp == 1))
                        ot = ob.tile([P, 512], F32, tag="ot",
                                     name=f"ot{tb}_{oc}")
                        nc.vector.tensor_copy(ot[:], pt[:])
                        nc.sync.dma_start(
                            out[tbs, oc * 512:(oc + 1) * 512], ot[:])

                for tt in range(NQT):
                    ts = slice(tt * 512, (tt + 1) * 512)
                    # Q/K projection + rope for this 512-token chunk
                    for proj, pair in (("q", 0), ("k", 0), ("q", 1), ("k", 1)):
                        psq = psQK.tile([P, 512], F32, tag="qk")
                        for dc in range(DC):
                            nc.tensor.matmul(
                                psq[:],
                                w_sb[proj][:, dc, pair * P:(pair + 1) * P],
                                xt_sb[:, dc, ts],
                                start=(dc == 0), stop=(dc == DC - 1))
                        qb = shp.tile([P, 512], BF16, tag="qb")
                        nc.vector.tensor_copy(qb[:], psq[:])
                        sh = shp.tile([P, 512], BF16, tag="sh")
                        nc.vector.stream_shuffle(sh[:], qb[:], XOR1)
                        # all rope muls on DVE: gpsimd must stay on a single
                        # ucode library (partition_broadcast) or it thrashes
                        # ~7us per library swap
                        sh2 = shp.tile([P, 512], BF16, tag="sh2")
                        nc.vector.tensor_mul(sh2[:], sh[:], sin_sb[:, ts])
                        dst = qk[(proj, pair)]
                        nc.vector.tensor_mul(dst[:, ts], qb[:], cos_sb[:, ts])
                        nc.vector.tensor_add(dst[:, ts], dst[:, ts], sh2[:])

                    # attention stage qt = tt (both pairs)
                    qt = tt
                    nkb = 4 * qt + 4
                    for pair in range(2):
                        qtile = qk[("q", pair)]
                        ktile = qk[("k", pair)]
                        avs = [psAV.tile([HD + 1, 512], F32, tag="av",
                                         name=f"av{pair}{qt}o{o}")
                               for o in range(2)]
                        def emit_av(pend):
                            pkb, poff, pw, et = pend
                            for o in range(2):
                                nc.tensor.matmul(
                                    avs[o][:, poff:512],
                                    vp[:, pkb, 2 * pair + o, :],
                                    et[:, 512 * o:512 * o + pw],
                                    start=(pkb == 0), stop=(pkb == nkb - 1),
                                    skip_group_check=True)

                        pend = None
                        for kb in range(nkb):
                            off = max(0, (kb - 4 * qt) * P)
                            w_ = 512 - off
                            diag = kb >= 4 * qt
                            # both heads side by side in one 2-bank tile so
                            # the exp can be a single ACT instruction
                            sc = psSC.tile([P, 1024], F32, tag="sc")
                            for o in range(2):
                                hs = slice(64 * o, 64 * o + 64)
                                base = 512 * o
                                klhs = ktile[hs, kb * P:(kb + 1) * P]
                                qs0 = qt * 512 + off
                                if diag:
                                    # poison the diagonal 128-col block with
                                    # -1e30 above the diagonal; score matmul
                                    # accumulates on top (has_written trick),
                                    # split at col 128 so each piece is
                                    # uniformly accumulate/overwrite.
                                    nc.tensor.matmul(
                                        sc[:, base:base + P], eyeneg[:],
                                        tri2[:], start=True, stop=True,
                                        skip_group_check=True)
                                    nc.tensor.matmul(
                                        sc[:, base:base + P], klhs,
                                        qtile[hs, qs0:qs0 + P],
                                        start=False, stop=True,
                                        skip_group_check=True)
                                    if w_ > P:
                                        nc.tensor.matmul(
                                            sc[:, base + P:base + w_], klhs,
                                            qtile[hs, qs0 + P:(qt + 1) * 512],
                                            start=False, stop=True,
                                            skip_group_check=True)
                                else:
                                    nc.tensor.matmul(
                                        sc[:, base:base + w_], klhs,
                                        qtile[hs, qs0:(qt + 1) * 512],
                                        start=True, stop=True,
                                        skip_group_check=True)
                            et = ex.tile([P, 1024], BF16, tag="et")
                            if w_ == 512:
                                nc.scalar.activation(
                                    et[:], sc[:], EXP, scale=0.125)
                            else:
                                for o in range(2):
                                    nc.scalar.activation(
                                        et[:, 512 * o:512 * o + w_],
                                        sc[:, 512 * o:512 * o + w_],
                                        EXP, scale=0.125)
                            if pend is not None:
                                emit_av(pend)
                                if opq:
                                    outproj_piece(opq.pop(0))
                            pend = (kb, off, w_, et)
                        emit_av(pend)
                        # normalize: yt = av * exp(-ln(den)); reciprocal via
                        # ACT ln/exp batched over the pair's two heads
                        for o in range(2):
                            nc.vector.tensor_copy(
                                den33[32 * o:32 * o + 1, :], avs[o][64:65, :])
                        nc.scalar.activation(lnd33[:], den33[:], LN)
                        nc.scalar.activation(rec33[:], lnd33[:], EXP,
                                             scale=-1.0)
                        # partition_broadcast ignores the AP base partition on
                        # HW (always reads physical partition 0): relocate
                        # head 1's row to its own tile first.
                        recb = nrm.tile([1, 512], F32, tag="recb")
                        nc.vector.tensor_copy(recb[:], rec33[32:33, :])
                        for o in range(2):
                            rb = nrm.tile([64, 512], F32, tag="rb")
                            nc.gpsimd.partition_broadcast(
                                rb[:], rec33[0:1, :] if o == 0 else recb[:])
                            nc.vector.tensor_mul(
                                yt[pair][64 * o:64 * o + 64, ts],
                                avs[o][0:64, :], rb[:])

                    # queue this qt's out-projection; emitted interleaved
                    # into the next stage's unit stream (PE gap filler)
                    opq.extend(range(4 * qt, 4 * qt + 4))
                for tb in opq:
                    outproj_piece(tb)

    nc.finalize()
    return nc


def _prep_core_inputs(x, pos, Wq, Wk, Wv, Wo):
    """Per-core input dicts (host-side sharding + layout prep)."""
    inv_freq = THETA ** (-np.arange(0, HD, 2, dtype=np.float32) / HD)
    ang = pos.astype(np.float32)[:, None] * inv_freq[None, :]   # (S, 32)
    cos = np.cos(ang).astype(np.float32)                        # (S, 32)
    sin = np.sin(ang).astype(np.float32)
    p = np.arange(P)
    pairidx = (p % HD) // 2
    cosP = np.ascontiguousarray(cos[:, pairidx].T)              # (128, S)
    sgn = np.where(p % 2 == 0, -1.0, 1.0).astype(np.float32)
    sinP = np.ascontiguousarray(sin[:, pairidx].T * sgn[:, None])

    bf = ml_dtypes.bfloat16
    cosPb = cosP.astype(bf)
    sinPb = sinP.astype(bf)
    xTs = [np.ascontiguousarray(x[b].T).astype(bf) for b in range(B)]  # (D, S)
    maps = []
    for c in range(NCORES):
        b, g = divmod(c, NH)
        cs = slice(C * g, C * (g + 1))
        maps.append({
            "xT": xTs[b],
            "wqT": np.ascontiguousarray(Wq[cs, :].T).astype(bf),
            "wkT": np.ascontiguousarray(Wk[cs, :].T).astype(bf),
            "wvT": np.ascontiguousarray(Wv[cs, :].T).astype(bf),
            "woT": np.ascontiguousarray(Wo[:, cs].T).astype(bf),
            "cosP": cosPb,
            "sinP": sinPb,
        })
    return maps


def kernel(in_features, token_positions, Wq, Wk, Wv, Wo):
    global _NC_CACHE, LAST_RESULTS
    x = np.asarray(in_features, dtype=np.float32)
    pos = np.asarray(token_positions)
    Wq = np.asarray(Wq, dtype=np.float32)
    Wk = np.asarray(Wk, dtype=np.float32)
    Wv = np.asarray(Wv, dtype=np.float32)
    Wo = np.asarray(Wo, dtype=np.float32)

    if _NC_CACHE is None:
        _NC_CACHE = _build()
    maps = _prep_core_inputs(x, pos, Wq, Wk, Wv, Wo)
    res = run_bass_kernel_spmd(_NC_CACHE, maps, core_ids=list(range(NCORES)))
    LAST_RESULTS = res
    parts = [r["out"] for r in res.results]
    outb = [parts[4 * b] + parts[4 * b + 1] + parts[4 * b + 2] + parts[4 * b + 3]
            for b in range(B)]
    return np.stack(outb).astype(np.float32)


if __name__ == "__main__":
    rng = np.random.default_rng(0)
    x = rng.standard_normal((B, S, D), dtype=np.float32)
    o = kernel(x, np.arange(S, dtype=np.int32),
               *(rng.standard_normal((D, D), dtype=np.float32) / 32
                 for _ in range(4)))
    print(o.shape, o.dtype)
